# revision 42
# baseline (speedup 1.0000x reference)
"""Trainium2 Bass kernel for nn_ConstrainedEnhancementModel.

Contract: kernel(**inputs) takes the FULL unsharded inputs (as produced by
reference.setup_inputs()) and returns the FULL [4096, 2000, 6] float32 output.

Strategy (pure data parallel over 8 NeuronCores, 512 batch rows each):
  - Feature-major MLP chain in fp8 (e4m3) with DoubleRow matmuls: weights are
    scaled x64 into fp8's normal range; b1/b4 are folded into constant-1 K
    rows; h2/h5 are stored pre-scaled x64 so every activation is a single
    scalar-ACT or single vector tensor_scalar op (alternating engines).
  - x arrives host-side pre-transposed twice: compact 5-ktile fp8 (for L1,
    packed with W1 in the wxa blob) and window-blocked bf16 (partition
    32w+r = x col 24*(4*i4+w)+r) in the per-group xbg blob for the
    interpolation matmul.
  - Final layer: out = h5 @ (W6 * c_d * 256 / 64) + x @ (G * 256), evaluated
    per output window (480 cols); fp8 DoubleRow pairs for the W6 part, a K=32
    bf16 matmul on a 32-row PE tile for the G (lin-interp + b6) part -- the
    four window interp matmuls run concurrently on disjoint 32-row strips.
    The psum->sbuf copy applies 1/256 and writes bf16; output DMAs one
    [128, nwin*480] chunk per (group, batch-tile).
  - Output tensor is bf16 (within the rel-err budget); host upcasts to f32.
  - Schedule: 12 warm-up matmuls keep the PE HAM clock at 2.4GHz while the
    wxa blob (big-row DMA) lands, and dummy filler matmuls bridge encoder
    layer-boundary waits so HAM never re-throttles; per-group xbg/w6 loads
    feed the final layer just-in-time (small group 6 last for a short tail).
    L3 is fused into L4 host-side (W34 = W3 @ W4_feat) since L3 is linear.
"""

import numpy as np
import ml_dtypes

import bass_rust
import concourse.bass as bass
import concourse.bacc as bacc
import concourse.mybir as mybir
import concourse.tile as tile
from concourse import bass_utils

F32 = mybir.dt.float32
BF16 = mybir.dt.bfloat16
F8 = mybir.dt.float8e4
BF16_NP = ml_dtypes.bfloat16
F8_NP = ml_dtypes.float8_e4m3fn

# Problem config (hardcoded; must match the reference)
LOW_T = 100
HIGH_T = 2000
FEAT = 6
HID = 256
NUM_CLASSES = 10
LBL_DIM = 16
UP = 20
B = 4096
NCORES = 8
BC = B // NCORES          # 512 batch rows per core
NBT = BC // 128           # 4 batch tiles per core
D_IN = LOW_T * FEAT       # 600
D_OUT = HIGH_T * FEAT     # 12000
NW = 25                   # output windows (80 timesteps * 6 feats = 480 cols)
WT = 480
NI4 = 7                   # ceil(25/4) groups of 4 windows
EW = 64.0                 # encoder weight fp8 scale
SC = 256.0                # decoder/W6/G fp8+psum scale
DR = mybir.MatmulPerfMode.DoubleRow
I4_ORDER = [0, 1, 2, 3, 4, 5, 6]   # small group last: shortest drain tail

# wxa1/wxa2 blobs (fp8): L1 inputs split so the first DoubleRow pair can
# start ~2us before the full blob lands.  wxa1 = x kt0,kt1 | W1 kt0,kt1;
# wxa2 = x kt2..kt4 | W1 kt2..kt4 (ktile k: x col 128k+p on partition p).
WXA1 = 2048
WXA2 = 3072
OW2 = 0            # W2: 4 ktiles x 256
OW34 = 1024        # W34 = W3@W4_feat fused: 2 ktiles x 256 (stored x2)
OW4B = 1536        # W4 emb part: rows 0-15 = 128*W4[128:144], row 16 = 128*b34
OW5 = 1792         # W5: 2 ktiles x 512
OL4E = 2816        # l4emb ktile (emb rows 0-15, const-1 row 16)
WXB = 3328


def _ap3(t, col_off, stride2, n3):
    """3-dim AP over all 128 partitions of tile t: [128, 2, n3]."""
    a = t[:]
    return bass_rust.AP(
        tensor=a.tensor, offset=a.offset + col_off,
        ap=[[a.ap[0][0], 128], [stride2, 2], [1, n3]],
    )


def _build_nc():
    """Build the single-core Bass program (SPMD: same program on all 8)."""
    nc = bacc.Bacc("TRN2", target_bir_lowering=False, debug=False)

    wxa1_d = nc.dram_tensor("wxa1", [128, WXA1], F8, kind="ExternalInput")
    wxa2_d = nc.dram_tensor("wxa2", [128, WXA2], F8, kind="ExternalInput")
    wxb_d = nc.dram_tensor("wxb", [128, WXB], F8, kind="ExternalInput")
    bias_d = nc.dram_tensor("biasb", [128, 26], F32, kind="ExternalInput")
    xbg_d = nc.dram_tensor("xbg", [128, NI4 * 992], BF16, kind="ExternalInput")
    w6_d = nc.dram_tensor("w6p", [128, NW * 4 * WT], F8, kind="ExternalInput")
    y_d = nc.dram_tensor("y", [BC, D_OUT], BF16, kind="ExternalOutput")

    RELU = mybir.ActivationFunctionType.Relu
    IDENT = mybir.ActivationFunctionType.Identity
    ADD = mybir.AluOpType.add
    MAX = mybir.AluOpType.max
    MULT = mybir.AluOpType.mult

    with tile.TileContext(nc) as tc:
        with (
            tc.tile_pool(name="const", bufs=1) as cp,
            tc.tile_pool(name="outpool", bufs=6) as op,
            tc.tile_pool(name="ppool", bufs=4, space="PSUM") as pm,
        ):
            # ---- persistent SBUF tensors ----
            wxa1 = cp.tile([128, WXA1], F8, tag="wxa1", name="wxa1")
            wxa2 = cp.tile([128, WXA2], F8, tag="wxa2", name="wxa2")
            wxb = cp.tile([128, WXB], F8, tag="wxb", name="wxb")
            cbias = cp.tile([128, 26], F32, tag="cbias", name="cbias")
            xbg = cp.tile([128, NI4 * 992], BF16, tag="xbg", name="xbg")
            w6all = cp.tile([128, NW * 4 * WT], F8, tag="w6all", name="w6all")
            h1a = cp.tile([128, 2 * BC], F8, tag="h1a", name="h1a")
            h1b = cp.tile([128, 2 * BC], F8, tag="h1b", name="h1b")
            h2 = cp.tile([128, 2 * BC], F8, tag="h2", name="h2")
            h4 = cp.tile([128, 2 * BC], F8, tag="h4", name="h4")
            h5a = cp.tile([128, 2 * BC], F8, tag="h5a", name="h5a")
            h5b = cp.tile([128, 2 * BC], F8, tag="h5b", name="h5b")
            dmy = cp.tile([128, 512], F8, tag="dmy", name="dmy")

            # warm-up operand: memset on the (otherwise idle) vector engine
            nc.vector.memset(dmy[:], 0.0)

            # ---- loads, ordered by first use ----
            nc.sync.dma_start(wxa1[:], wxa1_d[:])
            nc.sync.dma_start(wxa2[:], wxa2_d[:])
            nc.sync.dma_start(cbias[:], bias_d[:])
            nc.sync.dma_start(wxb[:], wxb_d[:])
            for g in I4_ORDER:
                nwin = 4 if g < 6 else 1
                nc.sync.dma_start(
                    xbg[:, 992 * g:992 * g + 992], xbg_d[:, 992 * g:992 * g + 992]
                )
                o = g * 4 * WT * 4
                nc.sync.dma_start(
                    w6all[:, o:o + nwin * 4 * WT], w6_d[:, o:o + nwin * 4 * WT]
                )

            # bias column views (f32): col 0 = 0.0, 17-18 = 64*b2,
            # 19 = b3 raw, 22-25 = 64*b5
            zb = cbias[:, 0:1]
            vb2 = [cbias[:, 17 + m:18 + m] for m in range(2)]
            vb3 = cbias[:, 19:20]
            vb5 = [cbias[:, 22 + m:23 + m] for m in range(4)]

            def act_s0(dst, ps):
                # relu(ps)/64 on the scalar engine (bias folded into matmul)
                nc.scalar.activation(dst, ps, RELU, bias=zb, scale=1.0 / EW)

            def act_v0(dst, ps):
                # relu(ps)/64 on the vector engine
                nc.vector.tensor_scalar(dst, ps, 0.0, 1.0 / EW, MAX, MULT)

            def act_sb(dst, ps, vb):
                # relu(ps + 64b): output stays pre-scaled x64
                nc.scalar.activation(dst, ps, RELU, bias=vb, scale=1.0)

            def act_vb(dst, ps, vb):
                nc.vector.tensor_scalar(dst, ps, vb, 0.0, ADD, MAX)

            def warm(n):
                # dummy matmuls: fill PE idle windows (act/DMA waits) so the
                # HAM activity monitor never re-throttles the clock to 1.2GHz.
                # One pool tile per call: the WAW chain needs no pool sems.
                psd = pm.tile([128, 1024], F32, tag="ps", name="ps")
                for _ in range(n):
                    nc.tensor.matmul(psd[:, 0:512], dmy[:, 0:128], dmy[:],
                                     start=True, stop=True)

            # ---- PE warm-up: sustained activity so HAM unthrottles to 2.4GHz
            # before L1 starts and while the wxa DMA lands (12 x N=512 cold ~= 5.1us)
            warm(12)

            # ---- encoder MLP (feature-major, fp8 DoubleRow) ----
            # L1: [600->512] compact x, 5 ktiles = 2 DR pairs + 1 plain.
            # All p0 matmuls run first (need only wxa1, which lands ~2us
            # before wxa2); p1+plain follow, then the act per m-block.
            h1t = [h1a, h1a, h1b, h1b]
            l1ps = []
            for m in range(4):
                ps = pm.tile([128, 1024], F32, tag="ps", name="ps")
                l1ps.append(ps)
                nc.tensor.matmul(
                    ps[:, 0:BC],
                    _ap3(wxa1, 1024 + m * 128, 512, 128),
                    _ap3(wxa1, 0, 512, 512),
                    start=True, stop=False, perf_mode=DR,
                )
            for m in range(4):
                ps = l1ps[m]
                nc.tensor.matmul(
                    ps[:, 0:BC],
                    _ap3(wxa2, 1536 + m * 128, 512, 128),
                    _ap3(wxa2, 0, 512, 512),
                    start=False, stop=False, perf_mode=DR,
                )
                nc.tensor.matmul(
                    ps[:, 0:BC],
                    wxa2[:, 2560 + m * 128:2560 + (m + 1) * 128],
                    wxa2[:, 1024:1536], start=False, stop=True,
                )
                dst = h1t[m][:, (m % 2) * BC:(m % 2 + 1) * BC]
                if m == 3:
                    # last act split across both engines: halves the latency
                    # on the L1 -> L2 critical boundary
                    nc.scalar.activation(h1t[3][:, BC:BC + 256], ps[:, 0:256],
                                         RELU, bias=zb, scale=1.0 / EW)
                    nc.vector.tensor_scalar(h1t[3][:, BC + 256:2 * BC],
                                            ps[:, 256:BC], 0.0, 1.0 / EW,
                                            MAX, MULT)
                elif m % 2 == 0:
                    act_s0(dst, ps[:, 0:BC])
                else:
                    act_v0(dst, ps[:, 0:BC])
            warm(3)
            # L2: [512->256], 4 ktiles = 2 DR pairs; h2 stored x64
            for m in range(2):
                ps = pm.tile([128, 1024], F32, tag="ps", name="ps")
                for p, hsrc in enumerate((h1a, h1b)):
                    nc.tensor.matmul(
                        ps[:, 0:BC],
                        _ap3(wxb, OW2 + 2 * p * 256 + m * 128, 256, 128),
                        _ap3(hsrc, 0, BC, 512),
                        start=(p == 0), stop=(p == 1), perf_mode=DR,
                    )
                if m == 0:
                    act_sb(h2[:, 0:BC], ps[:, 0:BC], vb2[m])
                else:
                    nc.scalar.activation(h2[:, BC:BC + 256], ps[:, 0:256],
                                         RELU, bias=vb2[m], scale=1.0)
                    nc.vector.tensor_scalar(h2[:, BC + 256:2 * BC],
                                            ps[:, 256:BC], vb2[m], 0.0,
                                            ADD, MAX)
            warm(3)
            # L4' = fused L3+L4: [256 h2 + 16 emb] -> 256.
            # psum = 128*(h2@W34 + emb@W4b + b34); the emb part is an
            # independent K=32 strip matmul (start=True), the fused W34
            # DoubleRow pair accumulates on top.
            for m in range(2):
                ps = pm.tile([128, 1024], F32, tag="ps", name="ps")
                nc.tensor.matmul(
                    ps[:, 0:BC],
                    wxb[0:32, OW4B + m * 128:OW4B + (m + 1) * 128],
                    wxb[0:32, OL4E:OL4E + BC],
                    start=True, stop=False, tile_position=(0, 0),
                )
                nc.tensor.matmul(
                    ps[:, 0:BC], _ap3(wxb, OW34 + m * 128, 256, 128),
                    _ap3(h2, 0, BC, 512),
                    start=False, stop=True, perf_mode=DR,
                )
                if m == 0:
                    nc.scalar.activation(h4[:, 0:BC], ps[:, 0:BC], RELU,
                                         bias=zb, scale=1.0 / 128.0)
                else:
                    nc.scalar.activation(h4[:, BC:BC + 256], ps[:, 0:256],
                                         RELU, bias=zb, scale=1.0 / 128.0)
                    nc.vector.tensor_scalar(h4[:, BC + 256:2 * BC],
                                            ps[:, 256:BC], 0.0, 1.0 / 128.0,
                                            MAX, MULT)
            warm(3)
            # L5: [256->512]; h5 stored x64 in split tiles
            h5t = [h5a, h5a, h5b, h5b]
            for m in range(4):
                ps = pm.tile([128, 1024], F32, tag="ps", name="ps")
                nc.tensor.matmul(
                    ps[:, 0:BC], _ap3(wxb, OW5 + m * 128, 512, 128),
                    _ap3(h4, 0, BC, 512),
                    start=True, stop=True, perf_mode=DR,
                )
                if m % 2 == 0:
                    act_sb(h5t[m][:, 0:BC], ps[:, 0:BC], vb5[m])
                else:
                    # split across engines: h5a/h5b ready sooner for the
                    # final-layer DoubleRow passes
                    nc.scalar.activation(h5t[m][:, BC:BC + 256], ps[:, 0:256],
                                         RELU, bias=vb5[m], scale=1.0)
                    nc.vector.tensor_scalar(h5t[m][:, BC + 256:2 * BC],
                                            ps[:, 256:BC], vb5[m], 0.0,
                                            ADD, MAX)

            warm(3)

            # ---- final layer + fused constraint epilogue ----
            # Per (group, batch-tile): two 2-bank psum tiles hold the 4
            # windows (w0,w1 in psA banks, w2,w3 in psB banks); the psum->ob
            # copy is ONE strided-AP op per psum tile (scalar: psA, vector:
            # psB) so each engine runs ~55% busy and never backpressures PE.
            for i4 in I4_ORDER:
                nwin = 4 if i4 < 6 else 1
                for bt in range(NBT):
                    psA = pm.tile([128, 1024], F32, tag="ps", name="ps")
                    psB = psA if nwin == 1 else \
                        pm.tile([128, 1024], F32, tag="ps", name="ps")
                    pst = [psA, psA, psB, psB]
                    pss = [pst[w][:, (w % 2) * 512:(w % 2) * 512 + WT]
                           for w in range(nwin)]
                    # interp strips sit BETWEEN the two w6 passes: the psum
                    # stop flag lands on the last full-row matmul, so the
                    # copies trigger ~2 matmul slots earlier
                    for w in range(nwin):
                        nc.tensor.matmul(
                            pss[w],
                            _ap3(h5a, bt * 128, BC, 128),
                            _ap3(w6all, (i4 * 4 + w) * 4 * WT, WT, WT),
                            start=True, stop=False, perf_mode=DR,
                        )
                    for w in range(nwin):
                        p0 = 32 * w
                        nc.tensor.matmul(
                            pss[w],
                            xbg[p0:p0 + 32, 992 * i4 + bt * 128:992 * i4 + bt * 128 + 128],
                            xbg[p0:p0 + 32, 992 * i4 + 512:992 * i4 + 992],
                            start=False, stop=False, tile_position=(p0, 0),
                        )
                    for w in range(nwin):
                        nc.tensor.matmul(
                            pss[w],
                            _ap3(h5b, bt * 128, BC, 128),
                            _ap3(w6all, (i4 * 4 + w) * 4 * WT + 2 * WT, WT, WT),
                            start=False, stop=True, perf_mode=DR,
                        )
                    ob = op.tile([128, nwin * WT], BF16, tag=f"ob{nwin}", name=f"ob{nwin}")
                    yrow = y_d[bt * 128:(bt + 1) * 128,
                               i4 * 4 * WT:i4 * 4 * WT + nwin * WT]
                    if nwin == 1:
                        nc.scalar.mul(ob[:], pss[0], 1.0 / SC)
                        nc.sync.dma_start(yrow, ob[:])
                    else:
                        # two half-copies + two half-DMAs: the w0w1 DMA
                        # overlaps the w2w3 copy, shortening the drain tail
                        nc.scalar.mul(_ap3(ob, 0, WT, WT),
                                      _ap3(psA, 0, 512, WT), 1.0 / SC)
                        nc.sync.dma_start(
                            y_d[bt * 128:(bt + 1) * 128,
                                i4 * 4 * WT:i4 * 4 * WT + 2 * WT],
                            ob[:, 0:2 * WT])
                        nc.vector.tensor_scalar_mul(_ap3(ob, 2 * WT, WT, WT),
                                                    _ap3(psB, 0, 512, WT), 1.0 / SC)
                        nc.sync.dma_start(
                            y_d[bt * 128:(bt + 1) * 128,
                                i4 * 4 * WT + 2 * WT:i4 * 4 * WT + 4 * WT],
                            ob[:, 2 * WT:4 * WT])

    nc.compile()
    return nc


def _host_prep(inputs):
    """Build per-core in_maps from the full inputs."""
    x_full = np.asarray(inputs["low_res_data"], np.float32).reshape(B, D_IN)
    labels = np.asarray(inputs["labels"]).astype(np.int64)
    emb = np.asarray(inputs["emb"], np.float32)
    W6 = np.asarray(inputs["W6"], np.float32)
    b6 = np.asarray(inputs["b6"], np.float32)

    # per-timestep blend coefficients (match the reference formulas)
    t = np.arange(HIGH_T)
    seg = np.clip(t // UP, 0, LOW_T - 2)
    alpha = ((t - seg * UP) / UP).astype(np.float64)
    is_anchor = (t % UP) == 0
    interior = t < (LOW_T - 1) * UP
    blendf = np.where(is_anchor, 1.0, np.where(interior, 0.8, 0.0))
    c_d = np.where(is_anchor, 0.0, np.where(interior, 0.2, 1.0))
    c_start = blendf * (1.0 - alpha) * SC
    c_end = blendf * alpha * SC

    # G matrix, window-blocked: [128, NI4*480]; window i at partition
    # offset 32*(i%4), col block i//4.  Rows r=0..29 <-> x col 24*i + r,
    # row 30 = bias row (pairs with the 1.0 row of the x layout).
    gmat = np.zeros((128, NI4 * WT), np.float64)
    for tt in range(HIGH_T):
        i, dt = divmod(tt, 80)
        i4, wpos = divmod(i, 4)
        p0 = 32 * wpos
        sl = seg[tt] - 4 * i
        for f in range(FEAT):
            col = i4 * WT + FEAT * dt + f
            gmat[p0 + FEAT * sl + f, col] += c_start[tt]
            gmat[p0 + FEAT * (sl + 1) + f, col] += c_end[tt]
            gmat[p0 + 30, col] = c_d[tt] * SC * np.float64(b6[FEAT * tt + f])
    gmat = gmat.astype(np.float32).astype(BF16_NP)

    # W6 blob: [128, 100*480] fp8; window i block at col (i4*4+w)*1920,
    # sub-blocks [k2][ko] of 480 cols = W6 ktile (2*k2+ko) for that window.
    # h5 arrives pre-scaled x64, so the fp8 weight carries c_d*SC/64.
    c_d_full = np.repeat(c_d, FEAT).astype(np.float32)
    w6s = (W6 * (c_d_full * SC / EW)[None, :]).astype(np.float32)
    w6r = w6s.reshape(4, 128, NW, WT)
    w6blob = np.zeros((128, NW * 4 * WT), np.float32)
    for i in range(NW):
        i4, w = divmod(i, 4)
        for kt in range(4):
            o = (i4 * 4 + w) * 4 * WT + kt * WT
            w6blob[:, o:o + WT] = w6r[kt, :, i, :]
    w6blob = w6blob.astype(F8_NP)

    # wxa1/wxa2 shared parts: W1 ktiles (x64) + folded b1 row
    W1 = np.asarray(inputs["W1"], np.float32)
    wxa1s = np.zeros((128, WXA1), np.float32)
    for kt in range(2):
        wxa1s[:, 1024 + kt * 512:1024 + (kt + 1) * 512] = \
            W1[128 * kt:128 * (kt + 1), :] * EW
    wxa2s = np.zeros((128, WXA2), np.float32)
    for kt in range(2, 5):
        nr = min(128, D_IN - 128 * kt)
        wxa2s[:nr, 1536 + (kt - 2) * 512:1536 + (kt - 1) * 512] = \
            W1[128 * kt:128 * kt + nr, :] * EW
    wxa2s[88, 2560:3072] = np.asarray(inputs["b1"], np.float32) * EW
    wxa1s = wxa1s.astype(F8_NP)
    wxa2s = wxa2s.astype(F8_NP)

    # wxb shared part: W2 (x64), fused W34 (x2), W4 emb part (x128), W5 (x64)
    wxbs = np.zeros((128, WXB), np.float32)
    W2 = np.asarray(inputs["W2"], np.float32) * EW
    for kt in range(4):
        wxbs[:, OW2 + kt * 256:OW2 + (kt + 1) * 256] = W2[kt * 128:(kt + 1) * 128]
    # fused L3+L4: W34 = W3 @ W4_feat, b34 = b3 @ W4_feat + b4
    W3 = np.asarray(inputs["W3"], np.float32)
    W4 = np.asarray(inputs["W4"], np.float32)
    W34 = (W3 @ W4[:128]) * 2.0
    b34 = np.asarray(inputs["b3"], np.float32) @ W4[:128] + \
        np.asarray(inputs["b4"], np.float32)
    for kt in range(2):
        wxbs[:, OW34 + kt * 256:OW34 + (kt + 1) * 256] = W34[kt * 128:(kt + 1) * 128]
    wxbs[0:16, OW4B:OW4B + 256] = W4[128:144] * 128.0
    wxbs[16, OW4B:OW4B + 256] = b34 * 128.0
    W5 = np.asarray(inputs["W5"], np.float32) * EW
    for kt in range(2):
        wxbs[:, OW5 + kt * 512:OW5 + (kt + 1) * 512] = W5[kt * 128:(kt + 1) * 128]
    wxbs = wxbs.astype(F8_NP)

    # bias blob [128, 26] f32: col 0 zero, 17-18 = 64*b2, 19 = b3, 22-25 = 64*b5
    biasb = np.zeros((128, 26), np.float32)
    biasb[:, 17:19] = np.asarray(inputs["b2"], np.float32).reshape(2, 128).T * EW
    biasb[:, 19] = np.asarray(inputs["b3"], np.float32)
    biasb[:, 22:26] = np.asarray(inputs["b5"], np.float32).reshape(4, 128).T * EW

    in_maps = []
    for c in range(NCORES):
        sl = slice(c * BC, (c + 1) * BC)
        xc = x_full[sl]  # [BC, 600]
        xw = np.zeros((128, NI4 * 512), np.float32)
        for i in range(NW):
            i4, wpos = divmod(i, 4)
            p0 = 32 * wpos
            ncols = min(30, D_IN - 24 * i)
            xw[p0:p0 + ncols, i4 * 512:i4 * 512 + BC] = xc[:, 24 * i:24 * i + ncols].T
            xw[p0 + 30, i4 * 512:i4 * 512 + BC] = 1.0
        # xbg blob: per-group blocks [x_g (512) | G_g (480)] for split loads
        xbg = np.zeros((128, NI4 * 992), BF16_NP)
        for g in range(NI4):
            xbg[:, 992 * g:992 * g + 512] = xw[:, 512 * g:512 * (g + 1)].astype(BF16_NP)
            xbg[:, 992 * g + 512:992 * (g + 1)] = gmat[:, WT * g:WT * (g + 1)]
        # wxa1/wxa2: compact x for L1 (ktile k = x cols 128k..128k+127) + W1
        wxa1 = wxa1s.copy()
        wxa2 = wxa2s.copy()
        x8c = np.zeros((128, 5 * 512), np.float32)
        for kt in range(5):
            nr = min(128, D_IN - 128 * kt)
            x8c[:nr, kt * 512:kt * 512 + BC] = xc[:, 128 * kt:128 * kt + nr].T
        x8c[88, 4 * 512:5 * 512] = 1.0  # b1 row
        x8c = x8c.astype(F8_NP)
        wxa1[:, 0:1024] = x8c[:, 0:1024]
        wxa2[:, 0:1536] = x8c[:, 1024:2560]
        # wxb: shared weights + per-core l4emb ktile
        wxb = wxbs.copy()
        l4emb = np.zeros((128, BC), np.float32)
        l4emb[0:LBL_DIM] = emb[labels[sl]].T
        l4emb[16] = 1.0  # b4 row
        wxb[:, OL4E:OL4E + BC] = l4emb.astype(F8_NP)
        m = {"biasb": biasb, "w6p": w6blob, "wxa1": wxa1, "wxa2": wxa2,
             "wxb": wxb, "xbg": xbg}
        in_maps.append(m)
    return in_maps


_NC_CACHE = None


def kernel(**inputs) -> np.ndarray:
    global _NC_CACHE
    if _NC_CACHE is None:
        _NC_CACHE = _build_nc()
    nc = _NC_CACHE
    in_maps = _host_prep(inputs)
    res = bass_utils.run_bass_kernel_spmd(nc, in_maps, core_ids=list(range(NCORES)))
    out = np.concatenate([res.results[c]["y"] for c in range(NCORES)], axis=0)
    return out.astype(np.float32).reshape(B, HIGH_T, FEAT)
